# revision 1
# baseline (speedup 1.0000x reference)
"""Trainium2 Bass kernel for nn_CQLoss (composite loss function).

Strategy: pure data parallel over batch dim (64 batches -> 8 per core).
Per core:
  - recon term: rows of [rzs | sqrt(w)*pts] (host-concatenated) gathered by
    `mapping` via indirect DMA straight from HBM — one gather per batch
    fetches both the recon and pts operands; subtract on VectorE, square +
    per-partition accumulate on ScalarE (chunks 0..3) / VectorE (chunk 4, so
    the ScalarE tail ends before the last gather lands).
  - pts term:  pts/pts_gt pre-scaled by sqrt(landmark weight) on the host;
    subtract + square (2x-mode tensor_tensor) + 4x-mode tensor_scalar
    accumulate, all on VectorE.
  - KL term:   ln(V*qy + V*eps) on ScalarE (== ln(qy+eps) - ln(1/V)),
    multiplied by qy (2x) and tensor_scalar-accumulated (4x) on VectorE
    (tensor_reduce is the slowest DVE op - 1x - so it is avoided).
  - best term: tiny; landmark index on the partition dim, host pre-scaled,
    zero-padded to 128 partitions.
The large tensors travel as bf16 (quantization contributes ~5e-5 relative
error on the final scalar; the kernel is HBM-bandwidth-bound so this halves
its runtime). Each core emits per-partition partial sums; the host does the
final (cheap) reduction in float64 and applies the global mean scalings.

Written in raw bass (explicit semaphores): this toolchain's codegen allows at
most one attached sync-wait per compute instruction, so waits are emitted as
standalone wait_ge ops. One semaphore per DMA (increments of concurrent DMAs
on a shared semaphore interleave, so intermediate wait values are racy), and
same-engine back-to-back RAW pairs get an explicit self-wait (engine
pipelines have no interlocks). All constants travel in a single packed DMA
(the int32 mapping rides bit-cast through the f32 pack). All batches are
SBUF-resident; compute is issued in multi-batch chunks with small final
chunks so the end-of-stream serial tail is short.
"""

import os
import sys

import numpy as np

for _p in ("/opt/trn_rl_repo", "/root/.axon_site/_ro/trn_rl_repo"):
    if os.path.isdir(_p) and _p not in sys.path:
        sys.path.insert(0, _p)

B, S, D, P, C, V = 64, 128, 2048, 118, 2, 512
PC = P * C  # 236
K = D + PC  # combined gather row width: 2284
N_CORES = 8
BL = B // N_CORES  # 8 batches per core
ALPHA, BETA, GAMMA, EPS = 10.0, 0.1, 1.0, 1e-20
MARKS = (0, 29, 88, 117)
# disk + ALPHA*landmark == (1/PC) * (sum d^2 + W_MARK * sum_marks d^2) per
# (b,s) row: ALPHA * PC / (len(MARKS)*C) = 10 * 236 / 8
W_MARK = ALPHA * PC / (len(MARKS) * C)  # 295.0

# recon chunking: (start_batch, n_batches) per chunk; small chunks last so the
# end-of-stream gather -> sub -> square chain is short
ZCH = [(0, 2), (2, 2), (4, 2), (6, 1), (7, 1)]

# packed const layout (f32 cols): 0..7 mapping (int32 bits), 8 ln bias,
# 9..24 sqrt(w)*best, 25..40 sqrt(w)*best_gt
NCONST = 9 + 4 * BL * C  # 41

_CACHE: dict = {}


def _build_bass(vector_dims: int):
    import concourse.bass as bass
    from concourse import mybir

    f32 = mybir.dt.float32
    bf16 = mybir.dt.bfloat16
    i32 = mybir.dt.int32
    Act = mybir.ActivationFunctionType
    Alu = mybir.AluOpType

    nc = bass.Bass()

    zs = nc.dram_tensor("zs", [BL * S, D], bf16, kind="ExternalInput")
    # gath rows: [rzs_row (D) | sqrt(w)*pts_row (PC)]
    gath = nc.dram_tensor("gath", [BL * S, K], bf16, kind="ExternalInput")
    ptsgt = nc.dram_tensor("ptsgt", [BL, S, PC], bf16, kind="ExternalInput")
    qy = nc.dram_tensor("qy", [BL, S, V], bf16, kind="ExternalInput")
    cpack = nc.dram_tensor("cpack", [S, NCONST], f32, kind="ExternalInput")
    # partials: cols 0..3 recon chunks 0..3 (ScalarE), col 4 best (ScalarE),
    # col 5 q*log, col 6 pts_h0, col 7 pts_h1, col 8 recon chunk 4 (VectorE)
    po = nc.dram_tensor("po", [S, 9], f32, kind="ExternalOutput")

    ln_scale = float(vector_dims)
    BC = BL * C  # 16

    # DVE op counts:
    #  1 sub_best, 2 mul_q, 3 tsacc_q, 4 sub_rz_c0, 5 sub_rz_c1, 6 sub_rz_c2,
    #  7 sub_pts_h0, 8 sqmul_pts_h0, 9 tsacc_pts_h0, 10 sub_rz_c3,
    #  11 sub_pts_h1, 12 sqmul_pts_h1, 13 tsacc_pts_h1, 14 sub_rz_c4,
    #  15 sqmul_rz_c4, 16 tsacc_rz_c4
    # ACT op counts:
    #  1 sq_best, 2 ln_all, 3..6 sq_rz_c0..c3
    DVE_N = 16
    ACT_N = 6

    from contextlib import ExitStack

    with ExitStack() as ctx:
        zs_t = ctx.enter_context(nc.sbuf_tensor([S, BL * D], bf16))
        gt_t = ctx.enter_context(nc.sbuf_tensor([S, BL * K], bf16))
        qy_t = ctx.enter_context(nc.sbuf_tensor([S, BL * V], bf16))
        lq_t = ctx.enter_context(nc.sbuf_tensor([S, BL * V], bf16))
        pg_t = ctx.enter_context(nc.sbuf_tensor([S, BL * PC], bf16))
        cp_t = ctx.enter_context(nc.sbuf_tensor([S, NCONST], f32))
        bd_t = ctx.enter_context(nc.sbuf_tensor([S, BC], f32))
        acc_t = ctx.enter_context(nc.sbuf_tensor([S, 9], f32))
        sem_cp = ctx.enter_context(nc.semaphore("sem_cp"))
        sem_zs = [
            ctx.enter_context(nc.semaphore(f"sem_zs{c}")) for c in range(len(ZCH))
        ]
        sem_g = [ctx.enter_context(nc.semaphore(f"sem_g{i}")) for i in range(BL)]
        sem_qy = ctx.enter_context(nc.semaphore("sem_qy"))
        sem_pg = ctx.enter_context(nc.semaphore("sem_pg"))
        sem_dve = ctx.enter_context(nc.semaphore("sem_dve"))
        sem_act = ctx.enter_context(nc.semaphore("sem_act"))
        sem_out = ctx.enter_context(nc.semaphore("sem_out"))
        block = ctx.enter_context(nc.Block())

        # 3D views: [s, batch, col]
        gt3 = gt_t[:].rearrange("s (b k) -> s b k", b=BL)
        zs3 = zs_t[:].rearrange("s (b d) -> s b d", b=BL)
        pg3 = pg_t[:].rearrange("s (b p) -> s b p", b=BL)
        map_i = cp_t[:, 0:BL].bitcast(i32)

        @block.sync
        def _(sync):
            sync.dma_start(out=cp_t[:], in_=cpack[:]).then_inc(sem_cp, 16)
            # zs chunk 0 and qy early; ptsgt mid; remaining zs chunks follow
            s0, n0 = ZCH[0]
            sync.dma_start(
                out=zs_t[:, s0 * D : (s0 + n0) * D], in_=zs[s0 * S : (s0 + n0) * S, :]
            ).then_inc(sem_zs[0], 16)
            sync.dma_start(
                out=qy_t[:], in_=qy[:, :, :].rearrange("b s v -> s b v")
            ).then_inc(sem_qy, 16)
            s1, n1 = ZCH[1]
            sync.dma_start(
                out=zs_t[:, s1 * D : (s1 + n1) * D], in_=zs[s1 * S : (s1 + n1) * S, :]
            ).then_inc(sem_zs[1], 16)
            sync.dma_start(
                out=pg_t[:], in_=ptsgt[:, :, :].rearrange("b s p -> s b p")
            ).then_inc(sem_pg, 16)
            # stagger the remaining zs chunks using earlier DMA completions as
            # release clocks, so the shared SDMA engines weave them between
            # the (compute-critical) gathers instead of ahead of all of them
            s2, n2 = ZCH[2]
            sync.wait_ge(sem_zs[0], 16)
            sync.dma_start(
                out=zs_t[:, s2 * D : (s2 + n2) * D], in_=zs[s2 * S : (s2 + n2) * S, :]
            ).then_inc(sem_zs[2], 16)
            s3, n3 = ZCH[3]
            sync.wait_ge(sem_qy, 16)
            sync.dma_start(
                out=zs_t[:, s3 * D : (s3 + n3) * D], in_=zs[s3 * S : (s3 + n3) * S, :]
            ).then_inc(sem_zs[3], 16)
            s4, n4 = ZCH[4]
            sync.wait_ge(sem_zs[1], 16)
            sync.dma_start(
                out=zs_t[:, s4 * D : (s4 + n4) * D], in_=zs[s4 * S : (s4 + n4) * S, :]
            ).then_inc(sem_zs[4], 16)
            sync.wait_ge(sem_act, ACT_N)
            sync.wait_ge(sem_dve, DVE_N)
            sync.dma_start(out=po[:], in_=acc_t[:]).then_inc(sem_out, 16)
            sync.wait_ge(sem_out, 16)

        @block.gpsimd
        def _(gpsimd):
            gpsimd.wait_ge(sem_cp, 16)  # mapping loaded
            for i in range(BL):
                gpsimd.indirect_dma_start(
                    out=gt_t[:, i * K : (i + 1) * K],
                    out_offset=None,
                    in_=gath[:],
                    in_offset=bass.IndirectOffsetOnAxis(
                        ap=map_i[:, i : i + 1], axis=0
                    ),
                ).then_inc(sem_g[i], 16)

        def sub_rz_chunk(c):
            s, n = ZCH[c]
            return nc.vector.tensor_sub(
                gt3[:, s : s + n, :D], gt3[:, s : s + n, :D], zs3[:, s : s + n, :]
            )

        def wait_rz_chunk(vector, c):
            s, n = ZCH[c]
            vector.wait_ge(sem_zs[c], 16)
            for k in range(n):
                vector.wait_ge(sem_g[s + k], 16)

        @block.vector
        def _(vector):
            # best term: bd = sqrt(w)*(best - best_gt)
            vector.wait_ge(sem_cp, 16)
            nc.vector.tensor_sub(
                bd_t[:], cp_t[:, 9 : 9 + BC], cp_t[:, 9 + BC : 9 + 2 * BC]
            ).then_inc(sem_dve, 1)  # 1
            # q-term runs before the first gather-gated sub: it only needs
            # ln_all, so it fills VectorE's early idle window
            vector.wait_ge(sem_act, 2)  # ln_all done
            nc.vector.tensor_mul(lq_t[:], qy_t[:], lq_t[:]).then_inc(sem_dve, 1)  # 2
            vector.wait_ge(sem_dve, 2)  # same-engine RAW: mul_q must retire
            nc.vector.tensor_scalar(
                out=lq_t[:],
                in0=lq_t[:],
                scalar1=1.0,
                scalar2=0.0,
                op0=Alu.mult,
                op1=Alu.add,
                accum_out=acc_t[:, 5:6],
            ).then_inc(sem_dve, 1)  # 3
            wait_rz_chunk(vector, 0)
            sub_rz_chunk(0).then_inc(sem_dve, 1)  # 4
            wait_rz_chunk(vector, 1)
            sub_rz_chunk(1).then_inc(sem_dve, 1)  # 5
            wait_rz_chunk(vector, 2)
            sub_rz_chunk(2).then_inc(sem_dve, 1)  # 6
            # pts half 0: d = xm - gt (in place), pg = d*d, 4x accum
            for i in range(4):
                vector.wait_ge(sem_g[i], 16)
            vector.wait_ge(sem_pg, 16)
            nc.vector.tensor_sub(
                gt3[:, 0:4, D:], gt3[:, 0:4, D:], pg3[:, 0:4, :]
            ).then_inc(sem_dve, 1)  # 7
            vector.wait_ge(sem_dve, 7)
            nc.vector.tensor_mul(
                pg3[:, 0:4, :], gt3[:, 0:4, D:], gt3[:, 0:4, D:]
            ).then_inc(sem_dve, 1)  # 8
            vector.wait_ge(sem_dve, 8)
            nc.vector.tensor_scalar(
                out=pg_t[:, : 4 * PC],
                in0=pg_t[:, : 4 * PC],
                scalar1=1.0,
                scalar2=0.0,
                op0=Alu.mult,
                op1=Alu.add,
                accum_out=acc_t[:, 6:7],
            ).then_inc(sem_dve, 1)  # 9
            wait_rz_chunk(vector, 3)
            sub_rz_chunk(3).then_inc(sem_dve, 1)  # 10
            # pts half 1 runs while the last zs chunk's DMA is in flight
            for i in range(4, 8):
                vector.wait_ge(sem_g[i], 16)
            nc.vector.tensor_sub(
                gt3[:, 4:8, D:], gt3[:, 4:8, D:], pg3[:, 4:8, :]
            ).then_inc(sem_dve, 1)  # 11
            vector.wait_ge(sem_dve, 11)
            nc.vector.tensor_mul(
                pg3[:, 4:8, :], gt3[:, 4:8, D:], gt3[:, 4:8, D:]
            ).then_inc(sem_dve, 1)  # 12
            vector.wait_ge(sem_dve, 12)
            nc.vector.tensor_scalar(
                out=pg_t[:, 4 * PC :],
                in0=pg_t[:, 4 * PC :],
                scalar1=1.0,
                scalar2=0.0,
                op0=Alu.mult,
                op1=Alu.add,
                accum_out=acc_t[:, 7:8],
            ).then_inc(sem_dve, 1)  # 13
            # recon chunk 4 squared on DVE (d^2 lands in the consumed zs
            # batch-7 slot)
            wait_rz_chunk(vector, 4)
            sub_rz_chunk(4).then_inc(sem_dve, 1)  # 14
            s4 = ZCH[4][0]
            vector.wait_ge(sem_dve, 14)
            nc.vector.tensor_mul(
                zs3[:, s4, :], gt3[:, s4, :D], gt3[:, s4, :D]
            ).then_inc(sem_dve, 1)  # 15
            vector.wait_ge(sem_dve, 15)
            nc.vector.tensor_scalar(
                out=zs3[:, s4, :],
                in0=zs3[:, s4, :],
                scalar1=1.0,
                scalar2=0.0,
                op0=Alu.mult,
                op1=Alu.add,
                accum_out=acc_t[:, 8:9],
            ).then_inc(sem_dve, 1)  # 16

        @block.scalar
        def _(scalar):
            # best term: acc_t[:, 4] = per-partition sum(bd^2)
            scalar.wait_ge(sem_dve, 1)
            nc.scalar.activation(
                bd_t[:], bd_t[:], Act.Square, accum_out=acc_t[:, 4:5]
            ).then_inc(sem_act, 1)  # 1
            scalar.wait_ge(sem_qy, 16)
            nc.scalar.activation(
                lq_t[:], qy_t[:], Act.Ln, bias=cp_t[:, 8:9], scale=ln_scale
            ).then_inc(sem_act, 1)  # 2
            dve_at = {0: 4, 1: 5, 2: 6, 3: 10}
            for c in range(4):
                s, n = ZCH[c]
                scalar.wait_ge(sem_dve, dve_at[c])
                nc.scalar.activation(
                    gt3[:, s : s + n, :D],
                    gt3[:, s : s + n, :D],
                    Act.Square,
                    accum_out=acc_t[:, c : c + 1],
                ).then_inc(sem_act, 1)  # 3..6

    return nc


def _get_nc(vector_dims: int):
    key = ("nc", vector_dims)
    if key not in _CACHE:
        _CACHE[key] = _build_bass(vector_dims)
    return _CACHE[key]


def _prepare(inputs):
    import ml_dtypes

    bf16 = ml_dtypes.bfloat16

    zs = np.asarray(inputs["zs"], dtype=np.float32)
    rzs = np.asarray(inputs["rzs"], dtype=np.float32)
    pts = np.asarray(inputs["pts"], dtype=np.float32)
    pts_gt = np.asarray(inputs["pts_gt"], dtype=np.float32)
    qy = np.asarray(inputs["qy"], dtype=np.float32)
    best = np.asarray(inputs["best"], dtype=np.float64)
    best_gt = np.asarray(inputs["best_gt"], dtype=np.float64)
    mapping = np.asarray(inputs["mapping"])
    vector_dims = int(np.asarray(inputs["vector_dims"]))

    # sqrt of landmark weights, applied on the host (exact in f64)
    w_p = np.ones(P, dtype=np.float64)
    w_p[list(MARKS)] += W_MARK
    w_sq = np.sqrt(w_p)  # (118,)
    wc = w_sq[None, None, :, None]  # broadcast over (B, S, P, C)

    zs_b = np.ascontiguousarray(zs.astype(bf16))
    qy_b = np.ascontiguousarray(qy.astype(bf16))
    ptsgt_b = np.ascontiguousarray((pts_gt * wc).astype(bf16))
    # combined gather source: [rzs | sqrt(w)*pts] per row
    gath_b = np.empty((B, S, K), dtype=bf16)
    gath_b[:, :, :D] = rzs.astype(bf16)
    gath_b[:, :, D:] = (pts * wc).astype(bf16).reshape(B, S, PC)
    best_w = (best * w_sq[None, :, None]).astype(np.float32)
    bestgt_w = (best_gt * w_sq[None, :, None]).astype(np.float32)

    base = (np.arange(BL, dtype=np.int32) * S)[:, None]  # absolute row offsets
    BC = BL * C

    in_maps = []
    for c in range(N_CORES):
        sl = slice(c * BL, (c + 1) * BL)
        map_abs = np.ascontiguousarray(
            (mapping[sl].astype(np.int32) + base).T
        )  # (S, BL)
        cpk = np.zeros((S, NCONST), dtype=np.float32)
        cpk[:, 0:BL] = map_abs.view(np.float32)
        cpk[:, BL] = np.float32(vector_dims * EPS)
        cpk[:P, 9 : 9 + BC] = best_w[sl].transpose(1, 0, 2).reshape(P, BC)
        cpk[:P, 9 + BC : 9 + 2 * BC] = bestgt_w[sl].transpose(1, 0, 2).reshape(P, BC)
        in_maps.append(
            {
                "zs": zs_b[sl].reshape(BL * S, D),
                "gath": gath_b[sl].reshape(BL * S, K),
                "ptsgt": ptsgt_b[sl].reshape(BL, S, PC),
                "qy": qy_b[sl],
                "cpack": cpk,
            }
        )
    return in_maps, vector_dims


def _combine(results) -> np.ndarray:
    s_pts = np.float64(0.0)
    s_kl = np.float64(0.0)
    s_best = np.float64(0.0)
    s_recon = np.float64(0.0)
    for r in results:
        por = r["po"].astype(np.float64)
        s_recon += por[:, 0:4].sum() + por[:, 8].sum()
        s_best += por[:, 4].sum()
        s_kl += por[:, 5].sum()
        s_pts += por[:, 6:8].sum()

    kld = s_kl / (B * S)
    recon = s_recon / (B * S * D)
    pts_term = s_pts / (B * S * PC)
    best_term = s_best / (B * PC)
    total = BETA * kld + GAMMA * recon + pts_term + best_term
    return np.float32(total)


def kernel(**inputs) -> np.ndarray:
    from concourse.bass_utils import run_bass_kernel_spmd

    in_maps, vector_dims = _prepare(inputs)
    nc = _get_nc(vector_dims)

    trace = os.environ.get("KERNEL_TRACE", "") == "1"
    res = run_bass_kernel_spmd(nc, in_maps, core_ids=list(range(N_CORES)), trace=trace)
    if trace and res.exec_time_ns is not None:
        print(f"HW exec time: {res.exec_time_ns} ns")
        if res.instructions_and_trace is not None:
            print(f"trace: {res.instructions_and_trace[1]}")

    return _combine(res.results)



# revision 6
# speedup vs baseline: 1.3158x; 1.3158x over previous
"""Trainium2 Bass kernel for nn_CQLoss (composite loss function).

Strategy: pure data parallel over batch dim (64 batches -> 8 per core).

All large tensors travel as fp8 e4m3 (TRN FP8_EXP4 == ml_dtypes.float8_e4m3,
max 240): the kernel is DMA-bound and the cost is out-side bytes, so fp8
halves the bf16 baseline's traffic.  Quantization contributes ~2e-3 relative
error on the final scalar (tolerance 2e-2).

Per core:
  - d = (rz|pts)[mapping] - (zs|pts_gt): the host packs rows [-zs | -pts_gt]
    (neg) and [rzs | pts] (gath, the gather source).  neg rows are DMA'd in
    plainly (HWDGE); the indirect gather then lands on top of them with
    compute_op=add, so the subtraction happens inside the DMA engines and no
    compute engine ever touches the full-size subtraction.
  - sum-of-squares via TensorE gram trick: matmul(chunk, chunk) accumulated
    into a PSUM region; the diagonal of the accumulated gram holds
    sum-over-partitions of squares per column, so trace == sum d^2.  PE does
    square+reduce at 1/128 DVE cost.  Regions (one PSUM bank, f32):
    recon [*,0:128], pts [*,128:256], kl [*,256:384], marks [0:8,384:392]
    (landmark P-indices are host-permuted to the front of the PC block),
    best [0:16,448:464].
  - KL: qt = V*qy (fp8), l = Ln(qt + V*eps) on ScalarE (fp8 out), then
    gram(qt_chunk, l_chunk): trace == sum qt*ln(qt).
  - best: f32 via cpack, sqrt(w)-prescaled on host, DVE sub + f32 gram.
  - extraction: DVE multiplies each gram region by an f32 diagonal mask and
    tensor_scalar-accumulates per partition into acc columns; host applies
    the exact per-term normalizations in f64.

Raw bass (explicit semaphores), same conventions as the bf16 baseline:
standalone wait_ge ops, one sem per DMA stream, explicit self-waits for
same-engine RAW pairs.
"""

import os
import sys

import numpy as np

for _p in ("/opt/trn_rl_repo", "/root/.axon_site/_ro/trn_rl_repo"):
    if os.path.isdir(_p) and _p not in sys.path:
        sys.path.insert(0, _p)

B, S, D, P, C, V = 64, 128, 2048, 118, 2, 512
PC = P * C  # 236
K = D + PC  # combined row width: 2284
N_CORES = 8
BL = B // N_CORES  # 8 batches per core
ALPHA, BETA, GAMMA, EPS = 10.0, 0.1, 1.0, 1e-20
MARKS = (0, 29, 88, 117)
NMARK = len(MARKS)  # 4 -> 8 cols (C=2), host-permuted to the front
W_MARK = ALPHA * PC / (NMARK * C)  # 295.0

# neg-chunk layout: (start_batch, n_batches)
NCH = [(0, 2), (2, 2), (4, 2), (6, 2)]

# cpack layout (f32 cols): 0..7 mapping (int32 bits), 8 ln bias,
# 9..24 sqrt(w)*best, 25..40 sqrt(w)*best_gt, 41..168 diag(1.0)
BC = BL * C  # 16
COL_MAP = 0
COL_BIAS = BL  # 8
COL_BEST = 9
COL_BESTGT = 9 + BC  # 25
COL_DIAG = 9 + 2 * BC  # 41
NCONST = COL_DIAG + 128  # 169

# psum region column offsets
R_RECON, R_PTS, R_KL, R_MARK, R_BEST = 0, 128, 256, 384, 448

_CACHE: dict = {}


def _build_bass(vector_dims: int):
    import concourse.bass as bass
    from concourse import mybir

    f32 = mybir.dt.float32
    fp8 = mybir.dt.float8e4
    i32 = mybir.dt.int32
    Act = mybir.ActivationFunctionType
    Alu = mybir.AluOpType

    nc = bass.Bass()

    neg = nc.dram_tensor("neg", [BL * S, K], fp8, kind="ExternalInput")
    gath = nc.dram_tensor("gath", [BL * S, K], fp8, kind="ExternalInput")
    qt = nc.dram_tensor("qt", [BL, S, V], fp8, kind="ExternalInput")
    cpack = nc.dram_tensor("cpack", [S, NCONST], f32, kind="ExternalInput")
    # acc cols: 0 recon, 1 pts, 2 kl, 3 marks (rows 0:8), 4 best (rows 0:16)
    po = nc.dram_tensor("po", [S, 5], f32, kind="ExternalOutput")

    from contextlib import ExitStack

    with ExitStack() as ctx:
        d_t = ctx.enter_context(nc.sbuf_tensor([S, BL * K], fp8))
        qt_t = ctx.enter_context(nc.sbuf_tensor([S, BL * V], fp8))
        l_t = ctx.enter_context(nc.sbuf_tensor([S, BL * V], fp8))
        cp_t = ctx.enter_context(nc.sbuf_tensor([S, NCONST], f32))
        bd_t = ctx.enter_context(nc.sbuf_tensor([S, BC], f32))
        ext_t = ctx.enter_context(nc.sbuf_tensor([S, 512], f32))
        acc_t = ctx.enter_context(nc.sbuf_tensor([S, 5], f32))
        ps = ctx.enter_context(nc.psum_tensor([S, 512], f32))

        sem_cp = ctx.enter_context(nc.semaphore("sem_cp"))
        sem_n = [
            ctx.enter_context(nc.semaphore(f"sem_n{c}")) for c in range(len(NCH))
        ]
        sem_qt = ctx.enter_context(nc.semaphore("sem_qt"))
        sem_g = [ctx.enter_context(nc.semaphore(f"sem_g{b}")) for b in range(BL)]
        sem_act = ctx.enter_context(nc.semaphore("sem_act"))
        sem_dve = ctx.enter_context(nc.semaphore("sem_dve"))
        sem_pe = ctx.enter_context(nc.semaphore("sem_pe"))
        sem_out = ctx.enter_context(nc.semaphore("sem_out"))
        block = ctx.enter_context(nc.Block())

        d3 = d_t[:].rearrange("s (b k) -> s b k", b=BL)
        map_i = cp_t[:, COL_MAP : COL_MAP + BL].bitcast(i32)
        diag = cp_t[:, COL_DIAG : COL_DIAG + 128]

        @block.sync
        def _(sync):
            sync.dma_start(out=cp_t[:], in_=cpack[:]).then_inc(sem_cp, 16)
            for c, (bs, nb) in enumerate(NCH):
                sync.dma_start(
                    out=d_t[:, bs * K : (bs + nb) * K],
                    in_=neg[bs * S : (bs + nb) * S, :],
                ).then_inc(sem_n[c], 16)
                if c == 1:
                    # qt rides between neg chunks so ACT can start early and
                    # the gathers (gated on neg + Pool DGE) have time to prep
                    sync.dma_start(
                        out=qt_t[:, : BL // 2 * V],
                        in_=qt[: BL // 2, :, :].rearrange("b s v -> s b v"),
                    ).then_inc(sem_qt, 16)
                    sync.dma_start(
                        out=qt_t[:, BL // 2 * V :],
                        in_=qt[BL // 2 :, :, :].rearrange("b s v -> s b v"),
                    ).then_inc(sem_qt, 16)
            sync.wait_ge(sem_dve, 11)
            sync.dma_start(out=po[:], in_=acc_t[:]).then_inc(sem_out, 16)
            sync.wait_ge(sem_out, 16)

        @block.gpsimd
        def _(gpsimd):
            gpsimd.wait_ge(sem_cp, 16)  # mapping loaded
            for b in range(BL):
                c = next(i for i, (bs, nb) in enumerate(NCH) if bs <= b < bs + nb)
                gpsimd.wait_ge(sem_n[c], 16)
                gpsimd.indirect_dma_start(
                    out=d3[:, b, :],
                    out_offset=None,
                    in_=gath[:],
                    in_offset=bass.IndirectOffsetOnAxis(
                        ap=map_i[:, b : b + 1], axis=0
                    ),
                    compute_op=Alu.add,
                ).then_inc(sem_g[b], 16)

        NRC = D // 128  # 16 recon grams per batch

        def d_grams(tensor, b):
            tensor.wait_ge(sem_g[b], 16)
            for i in range(NRC):
                nc.tensor.matmul(
                    out=ps[:, R_RECON : R_RECON + 128],
                    lhsT=d3[:, b, i * 128 : (i + 1) * 128],
                    rhs=d3[:, b, i * 128 : (i + 1) * 128],
                    start=(b == 0 and i == 0),
                    stop=(b == BL - 1 and i == NRC - 1),
                    skip_group_check=True,
                )
            nc.tensor.matmul(
                out=ps[:, R_PTS : R_PTS + 128],
                lhsT=d3[:, b, D : D + 128],
                rhs=d3[:, b, D : D + 128],
                start=(b == 0),
                stop=False,
                skip_group_check=True,
            )
            nc.tensor.matmul(
                out=ps[0 : PC - 128, R_PTS : R_PTS + PC - 128],
                lhsT=d3[:, b, D + 128 : D + PC],
                rhs=d3[:, b, D + 128 : D + PC],
                start=False,
                stop=(b == BL - 1),
                skip_group_check=True,
            )
            return nc.tensor.matmul(
                out=ps[0 : NMARK * C, R_MARK : R_MARK + NMARK * C],
                lhsT=d3[:, b, D : D + NMARK * C],
                rhs=d3[:, b, D : D + NMARK * C],
                start=(b == 0),
                stop=(b == BL - 1),
                skip_group_check=True,
            )

        @block.tensor
        def _(tensor):
            for b in range(6):
                d_grams(tensor, b)
            # kl grams: ln halves are long done by the time batch 5 is gathered
            tensor.wait_ge(sem_act, 2)
            for c in range(BL * V // 128):
                nc.tensor.matmul(
                    out=ps[:, R_KL : R_KL + 128],
                    lhsT=qt_t[:, c * 128 : (c + 1) * 128],
                    rhs=l_t[:, c * 128 : (c + 1) * 128],
                    start=(c == 0),
                    stop=(c == BL * V // 128 - 1),
                    skip_group_check=True,
                )
            # best gram (f32, prescaled): bd from DVE
            tensor.wait_ge(sem_dve, 1)
            nc.tensor.matmul(
                out=ps[0:BC, R_BEST : R_BEST + BC],
                lhsT=bd_t[:, :],
                rhs=bd_t[:, :],
                start=True,
                stop=True,
                skip_group_check=True,
            ).then_inc(sem_pe, 1)  # kl+best regions closed
            for b in range(6, BL):
                last = d_grams(tensor, b)
            last.then_inc(sem_pe, 1)  # all regions closed

        @block.scalar
        def _(scalar):
            scalar.wait_ge(sem_cp, 16)
            scalar.wait_ge(sem_qt, 16)
            nc.scalar.activation(
                l_t[:, : BL // 2 * V],
                qt_t[:, : BL // 2 * V],
                Act.Ln,
                bias=cp_t[:, COL_BIAS : COL_BIAS + 1],
                scale=1.0,
            ).then_inc(sem_act, 1)
            scalar.wait_ge(sem_qt, 32)
            nc.scalar.activation(
                l_t[:, BL // 2 * V :],
                qt_t[:, BL // 2 * V :],
                Act.Ln,
                bias=cp_t[:, COL_BIAS : COL_BIAS + 1],
                scale=1.0,
            ).then_inc(sem_act, 1)

        @block.vector
        def _(vector):
            # best diff: bd = sqrt(w)*(best - best_gt), f32
            vector.wait_ge(sem_cp, 16)
            nc.vector.tensor_sub(
                bd_t[:], cp_t[:, COL_BEST : COL_BEST + BC],
                cp_t[:, COL_BESTGT : COL_BESTGT + BC],
            ).then_inc(sem_dve, 1)  # 1

            def ext(region, width, nrows, col, wait_dve):
                nc.vector.tensor_mul(
                    ext_t[0:nrows, region : region + width],
                    ps[0:nrows, region : region + width],
                    diag[0:nrows, 0:width],
                ).then_inc(sem_dve, 1)
                vector.wait_ge(sem_dve, wait_dve)  # same-engine RAW
                nc.vector.tensor_scalar(
                    out=ext_t[0:nrows, region : region + width],
                    in0=ext_t[0:nrows, region : region + width],
                    scalar1=1.0,
                    scalar2=0.0,
                    op0=Alu.mult,
                    op1=Alu.add,
                    accum_out=acc_t[0:nrows, col : col + 1],
                ).then_inc(sem_dve, 1)

            # early group: kl + best close first
            vector.wait_ge(sem_pe, 1)
            ext(R_KL, 128, S, 2, 2)  # dve 2,3
            ext(R_BEST, BC, BC, 4, 4)  # dve 4,5
            # late group: recon/pts/marks close after the last batch
            vector.wait_ge(sem_pe, 2)
            ext(R_RECON, 128, S, 0, 6)  # dve 6,7
            ext(R_PTS, 128, S, 1, 8)  # dve 8,9
            ext(R_MARK, NMARK * C, NMARK * C, 3, 10)  # dve 10,11

    return nc


def _get_nc(vector_dims: int):
    key = ("nc", vector_dims)
    if key not in _CACHE:
        _CACHE[key] = _build_bass(vector_dims)
    return _CACHE[key]


def _prepare(inputs):
    import ml_dtypes

    fp8 = ml_dtypes.float8_e4m3

    zs = np.asarray(inputs["zs"], dtype=np.float32)
    rzs = np.asarray(inputs["rzs"], dtype=np.float32)
    pts = np.asarray(inputs["pts"], dtype=np.float32)
    pts_gt = np.asarray(inputs["pts_gt"], dtype=np.float32)
    qy = np.asarray(inputs["qy"], dtype=np.float32)
    best = np.asarray(inputs["best"], dtype=np.float64)
    best_gt = np.asarray(inputs["best_gt"], dtype=np.float64)
    mapping = np.asarray(inputs["mapping"])
    vector_dims = int(np.asarray(inputs["vector_dims"]))

    # landmark P-indices permuted to the front of the P axis
    perm = list(MARKS) + [p for p in range(P) if p not in MARKS]
    pts_p = pts[:, :, perm, :].reshape(B, S, PC)
    ptsgt_p = pts_gt[:, :, perm, :].reshape(B, S, PC)

    neg_b = np.empty((B, S, K), dtype=fp8)
    neg_b[:, :, :D] = (-zs).astype(fp8)
    neg_b[:, :, D:] = (-ptsgt_p).astype(fp8)
    gath_b = np.empty((B, S, K), dtype=fp8)
    gath_b[:, :, :D] = rzs.astype(fp8)
    gath_b[:, :, D:] = pts_p.astype(fp8)
    qt_b = np.ascontiguousarray((qy * vector_dims).astype(fp8))

    # sqrt of landmark weights for the best term (exact in f64)
    w_p = np.ones(P, dtype=np.float64)
    w_p[list(MARKS)] += W_MARK
    w_sq = np.sqrt(w_p)
    best_w = (best * w_sq[None, :, None]).astype(np.float32)
    bestgt_w = (best_gt * w_sq[None, :, None]).astype(np.float32)

    base = (np.arange(BL, dtype=np.int32) * S)[:, None]

    in_maps = []
    for c in range(N_CORES):
        sl = slice(c * BL, (c + 1) * BL)
        map_abs = np.ascontiguousarray(
            (mapping[sl].astype(np.int32) + base).T
        )  # (S, BL)
        cpk = np.zeros((S, NCONST), dtype=np.float32)
        cpk[:, COL_MAP : COL_MAP + BL] = map_abs.view(np.float32)
        cpk[:, COL_BIAS] = np.float32(vector_dims * EPS)
        cpk[:P, COL_BEST : COL_BEST + BC] = (
            best_w[sl].transpose(1, 0, 2).reshape(P, BC)
        )
        cpk[:P, COL_BESTGT : COL_BESTGT + BC] = (
            bestgt_w[sl].transpose(1, 0, 2).reshape(P, BC)
        )
        cpk[:, COL_DIAG : COL_DIAG + 128] = np.eye(S, 128, dtype=np.float32)
        in_maps.append(
            {
                "neg": neg_b[sl].reshape(BL * S, K),
                "gath": gath_b[sl].reshape(BL * S, K),
                "qt": qt_b[sl],
                "cpack": cpk,
            }
        )
    return in_maps, vector_dims


def _combine(results, vector_dims) -> np.ndarray:
    s_recon = np.float64(0.0)
    s_pts = np.float64(0.0)
    s_kl = np.float64(0.0)
    s_mark = np.float64(0.0)
    s_best = np.float64(0.0)
    for r in results:
        por = r["po"].astype(np.float64)
        s_recon += por[:, 0].sum()
        s_pts += por[:, 1].sum()
        s_kl += por[:, 2].sum()
        s_mark += por[: NMARK * C, 3].sum()
        s_best += por[:BC, 4].sum()

    kld = s_kl / (vector_dims * B * S)
    recon = s_recon / (B * S * D)
    pts_term = (s_pts + W_MARK * s_mark) / (B * S * PC)
    best_term = s_best / (B * PC)
    total = BETA * kld + GAMMA * recon + pts_term + best_term
    return np.float32(total)


def kernel(**inputs) -> np.ndarray:
    from concourse.bass_utils import run_bass_kernel_spmd

    in_maps, vector_dims = _prepare(inputs)
    nc = _get_nc(vector_dims)

    trace = os.environ.get("KERNEL_TRACE", "") == "1"
    res = run_bass_kernel_spmd(nc, in_maps, core_ids=list(range(N_CORES)), trace=trace)
    if trace and res.exec_time_ns is not None:
        print(f"HW exec time: {res.exec_time_ns} ns")
        if res.instructions_and_trace is not None:
            print(f"trace: {res.instructions_and_trace[1]}")

    return _combine(res.results, vector_dims)


# revision 9
# speedup vs baseline: 1.4087x; 1.0706x over previous
"""Trainium2 Bass kernel for nn_CQLoss (composite loss function).

Strategy: pure data parallel over batch dim (64 batches -> 8 per core).

All large tensors travel as fp8 e4m3 (TRN FP8_EXP4 == ml_dtypes.float8_e4m3):
the kernel is DMA-bound and the DMA cost is out-side bytes, so fp8 halves the
bf16 baseline's traffic.  Quantization contributes ~2e-3 relative error on
the final scalar (tolerance 2e-2).

Per core:
  - d = (rz|pts)[mapping] - (zs|pts_gt): the host packs [-zs | -pts_gt] rows
    pre-transposed to the exact SBUF layout [S, BL*K] (plain DMA pairs
    elements linearly in AP order, so identical shapes on both sides make
    the pairing trivially correct), and [rzs | pts] as the gather source.
    neg is DMA'd in plainly (HWDGE); the indirect gather lands on top with
    compute_op=add, so the subtraction happens inside the DMA engines.
  - sum-of-squares via TensorE gram trick: matmul(chunk, chunk) accumulated
    into a PSUM region; the diagonal of the accumulated gram holds
    sum-over-partitions of squares per column, so trace == sum d^2.  All
    regions live in ONE psum bank as ONE accumulation group (hw `start`
    zeroes the bank, so interleaved groups in a bank clobber each other;
    a single group accumulating at different addresses is safe):
    recon [*,0:128], pts [*,128:256], kl [*,256:384], markx [*,384:512]
    (landmark P-indices host-permuted to the front of the PC block; markx
    grams cols D..D+16 of every batch, mask selects the 8 mark cols).
  - KL: qt = V*qy (fp8, SBUF layout), l = Ln(qt + V*eps) on ScalarE (fp8
    out), then gram(qt_chunk, l_chunk): trace == sum qt*ln(qt).
  - best: f32 via cpack, sqrt(w)-prescaled, DVE sub + ScalarE Square with
    accum_out (tiny).
  - extraction: one DVE tensor_mul of the psum bank against an f32
    weight-diagonal mask (term normalizations baked in, DMA'd during the
    tail shadow), then one tensor_scalar accumulate into an acc column;
    host sums in f64.

Raw bass (explicit semaphores): standalone wait_ge ops, cumulative counts on
per-stream semaphores (DMAs of one stream complete in FIFO order), explicit
self-waits for same-engine RAW pairs.
"""

import os
import sys

import numpy as np

for _p in ("/opt/trn_rl_repo", "/root/.axon_site/_ro/trn_rl_repo"):
    if os.path.isdir(_p) and _p not in sys.path:
        sys.path.insert(0, _p)

B, S, D, P, C, V = 64, 128, 2048, 118, 2, 512
PC = P * C  # 236
K = D + PC  # combined row width: 2284
N_CORES = 8
BL = B // N_CORES  # 8 batches per core
ALPHA, BETA, GAMMA, EPS = 10.0, 0.1, 1.0, 1e-20
MARKS = (0, 29, 88, 117)
NMARK = len(MARKS)  # 4 -> 8 cols (C=2), host-permuted to the front
W_MARK = ALPHA * PC / (NMARK * C)  # 295.0

# neg-chunk layout: (start_batch, n_batches); small leading chunks so the
# first gathers can start early
NCH = [(0, 1), (1, 1), (2, 2), (4, 2), (6, 2)]


def _chunk_of(b):
    return next(i for i, (bs, nb) in enumerate(NCH) if bs <= b < bs + nb)


# cpack layout (f32 cols): 0..7 mapping (int32 bits), 8 ln bias,
# 9..24 sqrt(w)*best, 25..40 sqrt(w)*best_gt
BC = BL * C  # 16
COL_MAP = 0
COL_BIAS = BL  # 8
COL_BEST = 9
COL_BESTGT = 9 + BC  # 25
NCONST = 9 + 2 * BC  # 41

# psum region column offsets (single bank, single accumulation group)
R_RECON, R_PTS, R_KL, R_MARK = 0, 128, 256, 384

_CACHE: dict = {}


def _build_bass(vector_dims: int):
    import concourse.bass as bass
    from concourse import mybir

    f32 = mybir.dt.float32
    fp8 = mybir.dt.float8e4
    i32 = mybir.dt.int32
    Act = mybir.ActivationFunctionType
    Alu = mybir.AluOpType

    nc = bass.Bass()

    neg = nc.dram_tensor("neg", [S, BL * K], fp8, kind="ExternalInput")
    gath = nc.dram_tensor("gath", [BL * S, K], fp8, kind="ExternalInput")
    qt = nc.dram_tensor("qt", [S, BL * V], fp8, kind="ExternalInput")
    cpack = nc.dram_tensor("cpack", [S, NCONST], f32, kind="ExternalInput")
    wdiag = nc.dram_tensor("wdiag", [S, 512], f32, kind="ExternalInput")
    # acc col 0: weighted recon+pts+kl+marks; col 1 (rows 0:118): best
    po = nc.dram_tensor("po", [S, 2], f32, kind="ExternalOutput")

    from contextlib import ExitStack

    with ExitStack() as ctx:
        d_t = ctx.enter_context(nc.sbuf_tensor([S, BL * K], fp8))
        qt_t = ctx.enter_context(nc.sbuf_tensor([S, BL * V], fp8))
        l_t = ctx.enter_context(nc.sbuf_tensor([S, BL * V], fp8))
        cp_t = ctx.enter_context(nc.sbuf_tensor([S, NCONST], f32))
        wd_t = ctx.enter_context(nc.sbuf_tensor([S, 512], f32))
        bd_t = ctx.enter_context(nc.sbuf_tensor([S, BC], f32))
        ext_t = ctx.enter_context(nc.sbuf_tensor([S, 512], f32))
        acc_t = ctx.enter_context(nc.sbuf_tensor([S, 2], f32))
        ps = ctx.enter_context(nc.psum_tensor([S, 512], f32))

        sem_cp = ctx.enter_context(nc.semaphore("sem_cp"))
        sem_neg = ctx.enter_context(nc.semaphore("sem_neg"))
        sem_qt = ctx.enter_context(nc.semaphore("sem_qt"))
        sem_g = ctx.enter_context(nc.semaphore("sem_g"))
        sem_wd = ctx.enter_context(nc.semaphore("sem_wd"))
        sem_act = ctx.enter_context(nc.semaphore("sem_act"))
        sem_dve = ctx.enter_context(nc.semaphore("sem_dve"))
        sem_pe = ctx.enter_context(nc.semaphore("sem_pe"))
        sem_out = ctx.enter_context(nc.semaphore("sem_out"))
        block = ctx.enter_context(nc.Block())

        d3 = d_t[:].rearrange("s (b k) -> s b k", b=BL)
        map_i = cp_t[:, COL_MAP : COL_MAP + BL].bitcast(i32)

        @block.sync
        def _(sync):
            sync.dma_start(out=cp_t[:], in_=cpack[:]).then_inc(sem_cp, 16)
            for c, (bs, nb) in enumerate(NCH):
                sync.dma_start(
                    out=d_t[:, bs * K : (bs + nb) * K],
                    in_=neg[:, bs * K : (bs + nb) * K],
                ).then_inc(sem_neg, 16)
            sync.dma_start(
                out=qt_t[:, : BL // 2 * V], in_=qt[:, : BL // 2 * V]
            ).then_inc(sem_qt, 16)
            sync.dma_start(
                out=qt_t[:, BL // 2 * V :], in_=qt[:, BL // 2 * V :]
            ).then_inc(sem_qt, 16)
            sync.dma_start(out=wd_t[:], in_=wdiag[:]).then_inc(sem_wd, 16)
            sync.wait_ge(sem_dve, 3)
            sync.wait_ge(sem_act, 3)
            sync.dma_start(out=po[:], in_=acc_t[:]).then_inc(sem_out, 16)
            sync.wait_ge(sem_out, 16)

        @block.gpsimd
        def _(gpsimd):
            gpsimd.wait_ge(sem_cp, 16)  # mapping loaded
            for b in range(BL):
                gpsimd.wait_ge(sem_neg, 16 * (_chunk_of(b) + 1))
                gpsimd.indirect_dma_start(
                    out=d_t[:, b * K : (b + 1) * K],
                    out_offset=None,
                    in_=gath[:],
                    in_offset=bass.IndirectOffsetOnAxis(
                        ap=map_i[:, b : b + 1], axis=0
                    ),
                    compute_op=Alu.add,
                ).then_inc(sem_g, 16)

        NRC = D // 128  # 16 recon grams per batch
        KLC = BL * V // 128  # 32 kl grams
        # single accumulation group across the whole bank: only the very
        # first matmul has start=True, only the very last has stop=True
        first_mm = [True]

        def mm(out, lhsT, rhs, stop=False):
            inst = nc.tensor.matmul(
                out=out,
                lhsT=lhsT,
                rhs=rhs,
                start=first_mm[0],
                stop=stop,
                skip_group_check=True,
            )
            first_mm[0] = False
            return inst

        def d_grams(tensor, b):
            tensor.wait_ge(sem_g, 16 * (b + 1))
            for i in range(NRC):
                mm(
                    ps[:, R_RECON : R_RECON + 128],
                    d3[:, b, i * 128 : (i + 1) * 128],
                    d3[:, b, i * 128 : (i + 1) * 128],
                )
            mm(
                ps[:, R_PTS : R_PTS + 128],
                d3[:, b, D : D + 128],
                d3[:, b, D : D + 128],
            )
            mm(
                ps[0 : PC - 128, R_PTS : R_PTS + PC - 128],
                d3[:, b, D + 128 : D + PC],
                d3[:, b, D + 128 : D + PC],
            )

        @block.tensor
        def _(tensor):
            for b in range(6):
                d_grams(tensor, b)
            # kl grams: ln halves are done by the time batch 5 is gathered
            tensor.wait_ge(sem_act, 2)
            for c in range(KLC):
                mm(
                    ps[:, R_KL : R_KL + 128],
                    qt_t[:, c * 128 : (c + 1) * 128],
                    l_t[:, c * 128 : (c + 1) * 128],
                )
            for b in range(6, BL):
                d_grams(tensor, b)
            # markx: cols D..D+16 of every batch -> out rows b*16+c;
            # wdiag keeps only (p % 16) < 8 (the landmark cols)
            mm(
                ps[:, R_MARK : R_MARK + 128],
                d3[:, :, D : D + 16],
                d3[:, :, D : D + 16],
                stop=True,
            ).then_inc(sem_pe, 1)

        @block.scalar
        def _(scalar):
            scalar.wait_ge(sem_cp, 16)
            scalar.wait_ge(sem_qt, 16)
            nc.scalar.activation(
                l_t[:, : BL // 2 * V],
                qt_t[:, : BL // 2 * V],
                Act.Ln,
                bias=cp_t[:, COL_BIAS : COL_BIAS + 1],
                scale=1.0,
            ).then_inc(sem_act, 1)
            scalar.wait_ge(sem_qt, 32)
            nc.scalar.activation(
                l_t[:, BL // 2 * V :],
                qt_t[:, BL // 2 * V :],
                Act.Ln,
                bias=cp_t[:, COL_BIAS : COL_BIAS + 1],
                scale=1.0,
            ).then_inc(sem_act, 1)
            # best term: acc[0:118, 1] = per-partition sum(bd^2), in place
            scalar.wait_ge(sem_dve, 1)
            nc.scalar.activation(
                bd_t[0:P, :],
                bd_t[0:P, :],
                Act.Square,
                accum_out=acc_t[0:P, 1:2],
            ).then_inc(sem_act, 1)

        @block.vector
        def _(vector):
            # best diff: bd = sqrt(w)*(best - best_gt), f32
            vector.wait_ge(sem_cp, 16)
            nc.vector.tensor_sub(
                bd_t[:],
                cp_t[:, COL_BEST : COL_BEST + BC],
                cp_t[:, COL_BESTGT : COL_BESTGT + BC],
            ).then_inc(sem_dve, 1)  # 1
            # fused extraction: whole bank x weighted diagonal, then one accum
            vector.wait_ge(sem_wd, 16)
            vector.wait_ge(sem_pe, 1)
            nc.vector.tensor_mul(ext_t[:], ps[:, :], wd_t[:]).then_inc(
                sem_dve, 1
            )  # 2
            vector.wait_ge(sem_dve, 2)  # same-engine RAW
            nc.vector.tensor_scalar(
                out=ext_t[:],
                in0=ext_t[:],
                scalar1=1.0,
                scalar2=0.0,
                op0=Alu.mult,
                op1=Alu.add,
                accum_out=acc_t[:, 0:1],
            ).then_inc(sem_dve, 1)  # 3

    return nc


def _get_nc(vector_dims: int):
    key = ("nc", vector_dims)
    if key not in _CACHE:
        _CACHE[key] = _build_bass(vector_dims)
    return _CACHE[key]


def _prepare(inputs):
    import ml_dtypes

    fp8 = ml_dtypes.float8_e4m3

    zs = np.asarray(inputs["zs"], dtype=np.float32)
    rzs = np.asarray(inputs["rzs"], dtype=np.float32)
    pts = np.asarray(inputs["pts"], dtype=np.float32)
    pts_gt = np.asarray(inputs["pts_gt"], dtype=np.float32)
    qy = np.asarray(inputs["qy"], dtype=np.float32)
    best = np.asarray(inputs["best"], dtype=np.float64)
    best_gt = np.asarray(inputs["best_gt"], dtype=np.float64)
    mapping = np.asarray(inputs["mapping"])
    vector_dims = int(np.asarray(inputs["vector_dims"]))

    # landmark P-indices permuted to the front of the P axis
    perm = list(MARKS) + [p for p in range(P) if p not in MARKS]
    pts_p = pts[:, :, perm, :].reshape(B, S, PC)
    ptsgt_p = pts_gt[:, :, perm, :].reshape(B, S, PC)

    neg_b = np.empty((B, S, K), dtype=fp8)
    neg_b[:, :, :D] = (-zs).astype(fp8)
    neg_b[:, :, D:] = (-ptsgt_p).astype(fp8)
    gath_b = np.empty((B, S, K), dtype=fp8)
    gath_b[:, :, :D] = rzs.astype(fp8)
    gath_b[:, :, D:] = pts_p.astype(fp8)
    qt_b = (qy * vector_dims).astype(fp8)

    # sqrt of landmark weights for the best term (exact in f64)
    w_p = np.ones(P, dtype=np.float64)
    w_p[list(MARKS)] += W_MARK
    w_sq = np.sqrt(w_p)
    best_w = (best * w_sq[None, :, None]).astype(np.float32)
    bestgt_w = (best_gt * w_sq[None, :, None]).astype(np.float32)

    # weighted diagonal extraction mask (term normalizations baked in)
    wd = np.zeros((S, 512), dtype=np.float32)
    ii = np.arange(128)
    wd[ii, R_RECON + ii] = GAMMA / (B * S * D)
    wd[ii, R_PTS + ii] = 1.0 / (B * S * PC)
    wd[ii, R_KL + ii] = BETA / (vector_dims * B * S)
    # markx out row p = b*16 + c with c in 0..16; marks are c < 8
    wmark = np.where((ii % 16) < NMARK * C, W_MARK / (B * S * PC), 0.0)
    wd[ii, R_MARK + ii] = wmark

    base = (np.arange(BL, dtype=np.int32) * S)[:, None]

    in_maps = []
    for c in range(N_CORES):
        sl = slice(c * BL, (c + 1) * BL)
        map_abs = np.ascontiguousarray(
            (mapping[sl].astype(np.int32) + base).T
        )  # (S, BL)
        cpk = np.zeros((S, NCONST), dtype=np.float32)
        cpk[:, COL_MAP : COL_MAP + BL] = map_abs.view(np.float32)
        cpk[:, COL_BIAS] = np.float32(vector_dims * EPS)
        cpk[:P, COL_BEST : COL_BEST + BC] = (
            best_w[sl].transpose(1, 0, 2).reshape(P, BC)
        )
        cpk[:P, COL_BESTGT : COL_BESTGT + BC] = (
            bestgt_w[sl].transpose(1, 0, 2).reshape(P, BC)
        )
        in_maps.append(
            {
                # pre-transposed to the SBUF layout [S, BL*K]
                "neg": np.ascontiguousarray(
                    neg_b[sl].transpose(1, 0, 2).reshape(S, BL * K)
                ),
                "gath": gath_b[sl].reshape(BL * S, K),
                "qt": np.ascontiguousarray(
                    qt_b[sl].transpose(1, 0, 2).reshape(S, BL * V)
                ),
                "cpack": cpk,
                "wdiag": wd,
            }
        )
    return in_maps, vector_dims


def _combine(results) -> np.ndarray:
    total = np.float64(0.0)
    for r in results:
        por = r["po"].astype(np.float64)
        total += por[:, 0].sum()  # weighted recon+pts+kl+marks
        total += por[:P, 1].sum() / (B * PC)  # best
    return np.float32(total)


def kernel(**inputs) -> np.ndarray:
    from concourse.bass_utils import run_bass_kernel_spmd

    in_maps, vector_dims = _prepare(inputs)
    nc = _get_nc(vector_dims)

    trace = os.environ.get("KERNEL_TRACE", "") == "1"
    res = run_bass_kernel_spmd(nc, in_maps, core_ids=list(range(N_CORES)), trace=trace)
    if trace and res.exec_time_ns is not None:
        print(f"HW exec time: {res.exec_time_ns} ns")
        if res.instructions_and_trace is not None:
            print(f"trace: {res.instructions_and_trace[1]}")

    return _combine(res.results)


# revision 17
# speedup vs baseline: 1.7519x; 1.2437x over previous
"""Trainium2 Bass kernel for nn_CQLoss (composite loss function).

Strategy: pure data parallel over batch dim (64 batches -> 8 per core).

All large tensors travel as fp8 e4m3 (TRN FP8_EXP4 == ml_dtypes.float8_e4m3):
the kernel is DMA-bound and the DMA cost is out-side bytes, so fp8 halves the
bf16 baseline's traffic.  Quantization contributes ~2e-3 relative error on
the final scalar (tolerance 2e-2).

Per core:
  - d = (rz|pts)[mapping] - (zs|pts_gt): the host packs [-zs | -pts_gt] rows
    pre-transposed to the exact SBUF layout [S, BL*K] (plain DMA pairs
    elements linearly in AP order, so identical shapes on both sides make
    the pairing trivially correct), and [rzs | pts] as the gather source.
    neg is DMA'd in plainly (HWDGE); the indirect gather lands on top with
    compute_op=add, so the subtraction happens inside the DMA engines.
  - sum-of-squares via TensorE gram trick: matmul(chunk, chunk) accumulated
    into a PSUM region; the diagonal of the accumulated gram holds
    sum-over-partitions of squares per column, so trace == sum d^2.  All
    regions live in ONE psum bank as ONE accumulation group (hw `start`
    zeroes the bank, so interleaved groups in a bank clobber each other;
    a single group accumulating at different addresses is safe):
    recon [*,0:128], pts [*,128:256], kl [*,256:384], markx [*,384:512]
    (landmark P-indices host-permuted to the front of the PC block; markx
    grams cols D..D+16 of every batch, mask selects the 8 mark cols).
  - KL: qt = V*qy (fp8, SBUF layout), l = Ln(qt + V*eps) on ScalarE (fp8
    out), then gram(qt_chunk, l_chunk): trace == sum qt*ln(qt).
  - best: f32 via cpack, sqrt(w)-prescaled, DVE sub + ScalarE Square with
    accum_out (tiny).
  - extraction: one DVE tensor_mul of the psum bank against an f32
    weight-diagonal mask (term normalizations baked in, DMA'd during the
    tail shadow), then one tensor_scalar accumulate into an acc column;
    host sums in f64.

Raw bass (explicit semaphores): standalone wait_ge ops, cumulative counts on
per-stream semaphores (DMAs of one stream complete in FIFO order), explicit
self-waits for same-engine RAW pairs.
"""

import os
import sys

import numpy as np

for _p in ("/opt/trn_rl_repo", "/root/.axon_site/_ro/trn_rl_repo"):
    if os.path.isdir(_p) and _p not in sys.path:
        sys.path.insert(0, _p)

B, S, D, P, C, V = 64, 128, 2048, 118, 2, 512
PC = P * C  # 236
K = D + PC  # combined row width: 2284
N_CORES = 8
BL = B // N_CORES  # 8 batches per core
ALPHA, BETA, GAMMA, EPS = 10.0, 0.1, 1.0, 1e-20
MARKS = (0, 29, 88, 117)
NMARK = len(MARKS)  # 4 -> 8 cols (C=2), host-permuted to the front
W_MARK = ALPHA * PC / (NMARK * C)  # 295.0

# neg-chunk layout: (start_batch, n_batches); small leading chunks so the
# first gathers can start early
NCH = [(0, 1), (1, 1), (2, 2), (4, 2), (6, 2)]


def _chunk_of(b):
    return next(i for i, (bs, nb) in enumerate(NCH) if bs <= b < bs + nb)


# cpack layout (f32 cols): 0..7 mapping (int32 bits), 8 ln bias,
# 9..24 sqrt(w)*best, 25..40 sqrt(w)*best_gt
BC = BL * C  # 16
COL_MAP = 0
COL_BIAS = BL  # 8
COL_BEST = 9
COL_BESTGT = 9 + BC  # 25
NCONST = 9 + 2 * BC  # 41

# psum region column offsets (single bank, single accumulation group)
R_RECON, R_PTS, R_KL, R_MARK = 0, 128, 256, 384

# DoubleRow fp8 matmul: two 128-col k-tiles per pass (2x PE throughput)
USE_DR = True

# sync-engine DMA program: ("cp",) ("neg",chunk) ("qt",half) ("wd",)
# ("wg",val) = wait sem_g >= val, to let gather transfers interleave
SCHED = [
    ("neg", 0),
    ("neg", 1),
    ("cp",),
    ("neg", 2),
    ("neg", 3),
    ("neg", 4),
    ("qt", 0),
    ("qt", 1),
    ("wd",),
]

_CACHE: dict = {}


def _build_bass(vector_dims: int):
    import concourse.bass as bass
    from concourse import mybir

    f32 = mybir.dt.float32
    fp8 = mybir.dt.float8e4
    i32 = mybir.dt.int32
    Act = mybir.ActivationFunctionType
    Alu = mybir.AluOpType

    nc = bass.Bass()

    neg = nc.dram_tensor("neg", [S, BL * K], fp8, kind="ExternalInput")
    gath = nc.dram_tensor("gath", [BL * S, K], fp8, kind="ExternalInput")
    qt = nc.dram_tensor("qt", [S, BL * V], fp8, kind="ExternalInput")
    cpack = nc.dram_tensor("cpack", [S, NCONST], f32, kind="ExternalInput")
    wdiag = nc.dram_tensor("wdiag", [S, 512], f32, kind="ExternalInput")
    # acc col 0: weighted recon+pts+kl+marks; col 1 (rows 0:118): best
    po = nc.dram_tensor("po", [S, 2], f32, kind="ExternalOutput")

    from contextlib import ExitStack

    with ExitStack() as ctx:
        d_t = ctx.enter_context(nc.sbuf_tensor([S, BL * K], fp8))
        qt_t = ctx.enter_context(nc.sbuf_tensor([S, BL * V], fp8))
        l_t = ctx.enter_context(nc.sbuf_tensor([S, BL * V], fp8))
        cp_t = ctx.enter_context(nc.sbuf_tensor([S, NCONST], f32))
        wd_t = ctx.enter_context(nc.sbuf_tensor([S, 512], f32))
        bd_t = ctx.enter_context(nc.sbuf_tensor([S, BC], f32))
        ext_t = ctx.enter_context(nc.sbuf_tensor([S, 512], f32))
        acc_t = ctx.enter_context(nc.sbuf_tensor([S, 2], f32))
        ps = ctx.enter_context(nc.psum_tensor([S, 512], f32))

        sem_cp = ctx.enter_context(nc.semaphore("sem_cp"))
        sem_neg = ctx.enter_context(nc.semaphore("sem_neg"))
        sem_qt = ctx.enter_context(nc.semaphore("sem_qt"))
        sem_g = ctx.enter_context(nc.semaphore("sem_g"))
        sem_wd = ctx.enter_context(nc.semaphore("sem_wd"))
        sem_act = ctx.enter_context(nc.semaphore("sem_act"))
        sem_dve = ctx.enter_context(nc.semaphore("sem_dve"))
        sem_pe = ctx.enter_context(nc.semaphore("sem_pe"))
        sem_out = ctx.enter_context(nc.semaphore("sem_out"))
        block = ctx.enter_context(nc.Block(no_gpsimd_drain=True))

        d3 = d_t[:].rearrange("s (b k) -> s b k", b=BL)
        map_i = cp_t[:, COL_MAP : COL_MAP + BL].bitcast(i32)

        @block.sync
        def _(sync):
            qh = BL // 2 * V
            for tok in SCHED:
                kind = tok[0]
                if kind == "cp":
                    sync.dma_start(out=cp_t[:], in_=cpack[:]).then_inc(sem_cp, 16)
                elif kind == "neg":
                    bs, nb = NCH[tok[1]]
                    sync.dma_start(
                        out=d_t[:, bs * K : (bs + nb) * K],
                        in_=neg[:, bs * K : (bs + nb) * K],
                    ).then_inc(sem_neg, 16)
                elif kind == "qt":
                    sl = slice(0, qh) if tok[1] == 0 else slice(qh, BL * V)
                    sync.dma_start(out=qt_t[:, sl], in_=qt[:, sl]).then_inc(
                        sem_qt, 16
                    )
                elif kind == "wd":
                    sync.dma_start(out=wd_t[:], in_=wdiag[:]).then_inc(sem_wd, 16)
                elif kind == "wg":
                    sync.wait_ge(sem_g, tok[1])
            sync.wait_ge(sem_dve, 3)
            sync.wait_ge(sem_act, 3)
            sync.dma_start(out=po[:], in_=acc_t[:]).then_inc(sem_out, 16)
            sync.wait_ge(sem_out, 16)

        @block.gpsimd
        def _(gpsimd):
            gpsimd.wait_ge(sem_cp, 16)  # mapping loaded
            for b in range(BL):
                gpsimd.wait_ge(sem_neg, 16 * (_chunk_of(b) + 1))
                gpsimd.indirect_dma_start(
                    out=d_t[:, b * K : (b + 1) * K],
                    out_offset=None,
                    in_=gath[:],
                    in_offset=bass.IndirectOffsetOnAxis(
                        ap=map_i[:, b : b + 1], axis=0
                    ),
                    compute_op=Alu.add,
                ).then_inc(sem_g, 16)

        # single accumulation group across the whole bank: only the very
        # first matmul has start=True, only the very last has stop=True
        first_mm = [True]
        DR = mybir.MatmulPerfMode.DoubleRow

        def mm(out, lhsT, rhs, stop=False, dr=False):
            inst = nc.tensor.matmul(
                out=out,
                lhsT=lhsT,
                rhs=rhs,
                start=first_mm[0],
                stop=stop,
                perf_mode=DR if dr else None,
                skip_group_check=True,
            )
            first_mm[0] = False
            return inst

        def gram(region, a, b=None):
            # square-gram of columns `a` (or cross-gram a x b) into `region`;
            # USE_DR pairs two k-tiles per pass (DoubleRow)
            if USE_DR:
                w = a.shape[-1] // 2
                a2 = a.rearrange("s (t c) -> s t c", t=2)
                b2 = a2 if b is None else b.rearrange("s (t c) -> s t c", t=2)
                return mm(ps[0:w, region : region + w], a2, b2, dr=True)
            w = a.shape[-1]
            return mm(ps[0:w, region : region + w], a, a if b is None else b)

        def d_grams(tensor, b):
            tensor.wait_ge(sem_g, 16 * (b + 1))
            rw = 256 if USE_DR else 128
            for i in range(D // rw):
                gram(R_RECON, d3[:, b, i * rw : (i + 1) * rw])
            if USE_DR:
                gram(R_PTS, d3[:, b, D : D + PC])
            else:
                gram(R_PTS, d3[:, b, D : D + 128])
                gram(R_PTS, d3[:, b, D + 128 : D + PC])

        @block.tensor
        def _(tensor):
            for b in range(6):
                d_grams(tensor, b)
            # kl grams: ln halves are done by the time batch 5 is gathered
            kw = 256 if USE_DR else 128
            tensor.wait_ge(sem_act, 2)
            for c in range(BL * V // kw):
                gram(
                    R_KL,
                    qt_t[:, c * kw : (c + 1) * kw],
                    l_t[:, c * kw : (c + 1) * kw],
                )
            for b in range(6, BL):
                d_grams(tensor, b)
            # markx: cols D..D+16 of every batch -> out rows b*16+c;
            # wdiag keeps only (p % 16) < 8 (the landmark cols)
            mm(
                ps[:, R_MARK : R_MARK + 128],
                d3[:, :, D : D + 16],
                d3[:, :, D : D + 16],
                stop=True,
            ).then_inc(sem_pe, 1)

        @block.scalar
        def _(scalar):
            scalar.wait_ge(sem_cp, 16)
            scalar.wait_ge(sem_qt, 16)
            nc.scalar.activation(
                l_t[:, : BL // 2 * V],
                qt_t[:, : BL // 2 * V],
                Act.Ln,
                bias=cp_t[:, COL_BIAS : COL_BIAS + 1],
                scale=1.0,
            ).then_inc(sem_act, 1)
            scalar.wait_ge(sem_qt, 32)
            nc.scalar.activation(
                l_t[:, BL // 2 * V :],
                qt_t[:, BL // 2 * V :],
                Act.Ln,
                bias=cp_t[:, COL_BIAS : COL_BIAS + 1],
                scale=1.0,
            ).then_inc(sem_act, 1)
            # best term: acc[0:118, 1] = per-partition sum(bd^2), in place
            scalar.wait_ge(sem_dve, 1)
            nc.scalar.activation(
                bd_t[0:P, :],
                bd_t[0:P, :],
                Act.Square,
                accum_out=acc_t[0:P, 1:2],
            ).then_inc(sem_act, 1)

        @block.vector
        def _(vector):
            # best diff: bd = sqrt(w)*(best - best_gt), f32
            vector.wait_ge(sem_cp, 16)
            nc.vector.tensor_sub(
                bd_t[:],
                cp_t[:, COL_BEST : COL_BEST + BC],
                cp_t[:, COL_BESTGT : COL_BESTGT + BC],
            ).then_inc(sem_dve, 1)  # 1
            # fused extraction: whole bank x weighted diagonal, then one accum
            vector.wait_ge(sem_wd, 16)
            vector.wait_ge(sem_pe, 1)
            nc.vector.tensor_mul(ext_t[:], ps[:, :], wd_t[:]).then_inc(
                sem_dve, 1
            )  # 2
            vector.wait_ge(sem_dve, 2)  # same-engine RAW
            nc.vector.tensor_scalar(
                out=ext_t[:],
                in0=ext_t[:],
                scalar1=1.0,
                scalar2=0.0,
                op0=Alu.mult,
                op1=Alu.add,
                accum_out=acc_t[:, 0:1],
            ).then_inc(sem_dve, 1)  # 3

    return nc


def _get_nc(vector_dims: int):
    key = ("nc", vector_dims)
    if key not in _CACHE:
        _CACHE[key] = _build_bass(vector_dims)
    return _CACHE[key]


def _prepare(inputs):
    import ml_dtypes

    fp8 = ml_dtypes.float8_e4m3

    zs = np.asarray(inputs["zs"], dtype=np.float32)
    rzs = np.asarray(inputs["rzs"], dtype=np.float32)
    pts = np.asarray(inputs["pts"], dtype=np.float32)
    pts_gt = np.asarray(inputs["pts_gt"], dtype=np.float32)
    qy = np.asarray(inputs["qy"], dtype=np.float32)
    best = np.asarray(inputs["best"], dtype=np.float64)
    best_gt = np.asarray(inputs["best_gt"], dtype=np.float64)
    mapping = np.asarray(inputs["mapping"])
    vector_dims = int(np.asarray(inputs["vector_dims"]))

    # landmark P-indices permuted to the front of the P axis
    perm = list(MARKS) + [p for p in range(P) if p not in MARKS]
    pts_p = pts[:, :, perm, :].reshape(B, S, PC)
    ptsgt_p = pts_gt[:, :, perm, :].reshape(B, S, PC)

    neg_b = np.empty((B, S, K), dtype=fp8)
    neg_b[:, :, :D] = (-zs).astype(fp8)
    neg_b[:, :, D:] = (-ptsgt_p).astype(fp8)
    gath_b = np.empty((B, S, K), dtype=fp8)
    gath_b[:, :, :D] = rzs.astype(fp8)
    gath_b[:, :, D:] = pts_p.astype(fp8)
    qt_b = (qy * vector_dims).astype(fp8)

    # sqrt of landmark weights for the best term (exact in f64)
    w_p = np.ones(P, dtype=np.float64)
    w_p[list(MARKS)] += W_MARK
    w_sq = np.sqrt(w_p)
    best_w = (best * w_sq[None, :, None]).astype(np.float32)
    bestgt_w = (best_gt * w_sq[None, :, None]).astype(np.float32)

    # weighted diagonal extraction mask (term normalizations baked in)
    wd = np.zeros((S, 512), dtype=np.float32)
    ii = np.arange(128)
    wd[ii, R_RECON + ii] = GAMMA / (B * S * D)
    wd[ii, R_PTS + ii] = 1.0 / (B * S * PC)
    wd[ii, R_KL + ii] = BETA / (vector_dims * B * S)
    # markx out row p = b*16 + c with c in 0..16; marks are c < 8
    wmark = np.where((ii % 16) < NMARK * C, W_MARK / (B * S * PC), 0.0)
    wd[ii, R_MARK + ii] = wmark

    base = (np.arange(BL, dtype=np.int32) * S)[:, None]

    in_maps = []
    for c in range(N_CORES):
        sl = slice(c * BL, (c + 1) * BL)
        map_abs = np.ascontiguousarray(
            (mapping[sl].astype(np.int32) + base).T
        )  # (S, BL)
        cpk = np.zeros((S, NCONST), dtype=np.float32)
        cpk[:, COL_MAP : COL_MAP + BL] = map_abs.view(np.float32)
        cpk[:, COL_BIAS] = np.float32(vector_dims * EPS)
        cpk[:P, COL_BEST : COL_BEST + BC] = (
            best_w[sl].transpose(1, 0, 2).reshape(P, BC)
        )
        cpk[:P, COL_BESTGT : COL_BESTGT + BC] = (
            bestgt_w[sl].transpose(1, 0, 2).reshape(P, BC)
        )
        in_maps.append(
            {
                # pre-transposed to the SBUF layout [S, BL*K]
                "neg": np.ascontiguousarray(
                    neg_b[sl].transpose(1, 0, 2).reshape(S, BL * K)
                ),
                "gath": gath_b[sl].reshape(BL * S, K),
                "qt": np.ascontiguousarray(
                    qt_b[sl].transpose(1, 0, 2).reshape(S, BL * V)
                ),
                "cpack": cpk,
                "wdiag": wd,
            }
        )
    return in_maps, vector_dims


def _combine(results) -> np.ndarray:
    total = np.float64(0.0)
    for r in results:
        por = r["po"].astype(np.float64)
        total += por[:, 0].sum()  # weighted recon+pts+kl+marks
        total += por[:P, 1].sum() / (B * PC)  # best
    return np.float32(total)


def kernel(**inputs) -> np.ndarray:
    from concourse.bass_utils import run_bass_kernel_spmd

    in_maps, vector_dims = _prepare(inputs)
    nc = _get_nc(vector_dims)

    trace = os.environ.get("KERNEL_TRACE", "") == "1"
    res = run_bass_kernel_spmd(nc, in_maps, core_ids=list(range(N_CORES)), trace=trace)
    if trace and res.exec_time_ns is not None:
        print(f"HW exec time: {res.exec_time_ns} ns")
        if res.instructions_and_trace is not None:
            print(f"trace: {res.instructions_and_trace[1]}")

    return _combine(res.results)


# revision 19
# speedup vs baseline: 1.7919x; 1.0228x over previous
"""Trainium2 Bass kernel for nn_CQLoss (composite loss function).

Strategy: pure data parallel over batch dim (64 batches -> 8 per core).

All large tensors travel as fp8 e4m3 (TRN FP8_EXP4 == ml_dtypes.float8_e4m3):
the kernel is DMA-bound and the DMA cost is out-side bytes, so fp8 halves the
bf16 baseline's traffic.  Quantization contributes ~2e-3 relative error on
the final scalar (tolerance 2e-2).

Per core:
  - d = (rz|pts)[mapping] - (zs|pts_gt): the host packs [-zs | -pts_gt] rows
    pre-transposed to the exact SBUF layout [S, BL*K] (plain DMA pairs
    elements linearly in AP order, so identical shapes on both sides make
    the pairing trivially correct), and [rzs | pts] as the gather source.
    neg is DMA'd in plainly (HWDGE); the indirect gather lands on top with
    compute_op=add, so the subtraction happens inside the DMA engines.
  - sum-of-squares via TensorE gram trick: matmul(chunk, chunk) accumulated
    into a PSUM region; the diagonal of the accumulated gram holds
    sum-over-partitions of squares per column, so trace == sum d^2.  All
    regions live in ONE psum bank as ONE accumulation group (hw `start`
    zeroes the bank, so interleaved groups in a bank clobber each other;
    a single group accumulating at different addresses is safe):
    recon [*,0:128], pts [*,128:256], kl [*,256:384], markx [*,384:512]
    (landmark P-indices host-permuted to the front of the PC block; markx
    grams cols D..D+16 of every batch, mask selects the 8 mark cols).
  - KL: qt = V*qy (fp8, SBUF layout), l = Ln(qt + V*eps) on ScalarE (fp8
    out), then gram(qt_chunk, l_chunk): trace == sum qt*ln(qt).
  - best: f32 via cpack, sqrt(w)-prescaled, DVE sub + ScalarE Square with
    accum_out (tiny).
  - extraction: one DVE tensor_mul of the psum bank against an f32
    weight-diagonal mask (term normalizations baked in, DMA'd during the
    tail shadow), then one tensor_scalar accumulate into an acc column;
    host sums in f64.

Raw bass (explicit semaphores): standalone wait_ge ops, cumulative counts on
per-stream semaphores (DMAs of one stream complete in FIFO order), explicit
self-waits for same-engine RAW pairs.
"""

import os
import sys

import numpy as np

for _p in ("/opt/trn_rl_repo", "/root/.axon_site/_ro/trn_rl_repo"):
    if os.path.isdir(_p) and _p not in sys.path:
        sys.path.insert(0, _p)

B, S, D, P, C, V = 64, 128, 2048, 118, 2, 512
PC = P * C  # 236
K = D + PC  # combined row width: 2284
N_CORES = 8
BL = B // N_CORES  # 8 batches per core
ALPHA, BETA, GAMMA, EPS = 10.0, 0.1, 1.0, 1e-20
MARKS = (0, 29, 88, 117)
NMARK = len(MARKS)  # 4 -> 8 cols (C=2), host-permuted to the front
W_MARK = ALPHA * PC / (NMARK * C)  # 295.0

# neg-chunk layout: (start_batch, n_batches); small leading chunks so the
# first gathers can start early
NCH = [(0, 1), (1, 1), (2, 2), (4, 2), (6, 2)]


def _chunk_of(b):
    return next(i for i, (bs, nb) in enumerate(NCH) if bs <= b < bs + nb)


# cpack layout (f32 cols): 0..7 mapping (int32 bits), 8 ln bias,
# 9..24 sqrt(w)*best, 25..40 sqrt(w)*best_gt
BC = BL * C  # 16
COL_MAP = 0
COL_BIAS = BL  # 8
COL_BEST = 9
COL_BESTGT = 9 + BC  # 25
NCONST = 9 + 2 * BC  # 41

# psum region column offsets (single bank, single accumulation group)
R_RECON, R_PTS, R_KL, R_MARK = 0, 128, 256, 384

# DoubleRow fp8 matmul: two 128-col k-tiles per pass (2x PE throughput)
USE_DR = True

# sync-engine DMA program: ("cp",) ("neg",chunk) ("qt",half) ("wd",)
# ("wg",val) = wait sem_g >= val, to let gather transfers interleave
SCHED = [
    ("neg", 0),
    ("neg", 1),
    ("cp",),
    ("neg", 2),
    ("neg", 3),
    ("neg", 4),
    ("qt", 0),
    ("qt", 1),
    ("wd",),
]

_CACHE: dict = {}


def _build_bass(vector_dims: int):
    import concourse.bass as bass
    from concourse import mybir

    f32 = mybir.dt.float32
    fp8 = mybir.dt.float8e4
    i32 = mybir.dt.int32
    Act = mybir.ActivationFunctionType
    Alu = mybir.AluOpType

    nc = bass.Bass()

    neg = nc.dram_tensor("neg", [S, BL * K], fp8, kind="ExternalInput")
    gath = nc.dram_tensor("gath", [BL * S, K], fp8, kind="ExternalInput")
    qt = nc.dram_tensor("qt", [S, BL * V], fp8, kind="ExternalInput")
    cpack = nc.dram_tensor("cpack", [S, NCONST], f32, kind="ExternalInput")
    wdiag = nc.dram_tensor("wdiag", [S, 512], f32, kind="ExternalInput")
    # acc col 0: weighted recon+pts+kl+marks; col 1 (rows 0:118): best
    po = nc.dram_tensor("po", [S, 2], f32, kind="ExternalOutput")

    from contextlib import ExitStack

    with ExitStack() as ctx:
        d_t = ctx.enter_context(nc.sbuf_tensor([S, BL * K], fp8))
        qt_t = ctx.enter_context(nc.sbuf_tensor([S, BL * V], fp8))
        l_t = ctx.enter_context(nc.sbuf_tensor([S, BL * V], fp8))
        cp_t = ctx.enter_context(nc.sbuf_tensor([S, NCONST], f32))
        wd_t = ctx.enter_context(nc.sbuf_tensor([S, 512], f32))
        bd_t = ctx.enter_context(nc.sbuf_tensor([S, BC], f32))
        ext_t = ctx.enter_context(nc.sbuf_tensor([S, 512], f32))
        acc_t = ctx.enter_context(nc.sbuf_tensor([S, 2], f32))
        ps = ctx.enter_context(nc.psum_tensor([S, 512], f32))

        sem_cp = ctx.enter_context(nc.semaphore("sem_cp"))
        sem_neg = ctx.enter_context(nc.semaphore("sem_neg"))
        sem_qt = ctx.enter_context(nc.semaphore("sem_qt"))
        sem_g = ctx.enter_context(nc.semaphore("sem_g"))
        sem_wd = ctx.enter_context(nc.semaphore("sem_wd"))
        sem_act = ctx.enter_context(nc.semaphore("sem_act"))
        sem_dve = ctx.enter_context(nc.semaphore("sem_dve"))
        sem_pe = ctx.enter_context(nc.semaphore("sem_pe"))
        sem_out = ctx.enter_context(nc.semaphore("sem_out"))
        block = ctx.enter_context(nc.Block(no_gpsimd_drain=True))

        d3 = d_t[:].rearrange("s (b k) -> s b k", b=BL)
        map_i = cp_t[:, COL_MAP : COL_MAP + BL].bitcast(i32)

        @block.sync
        def _(sync):
            qh = BL // 2 * V
            for tok in SCHED:
                kind = tok[0]
                if kind == "cp":
                    sync.dma_start(out=cp_t[:], in_=cpack[:]).then_inc(sem_cp, 16)
                elif kind == "neg":
                    bs, nb = NCH[tok[1]]
                    sync.dma_start(
                        out=d_t[:, bs * K : (bs + nb) * K],
                        in_=neg[:, bs * K : (bs + nb) * K],
                    ).then_inc(sem_neg, 16)
                elif kind == "qt":
                    sl = slice(0, qh) if tok[1] == 0 else slice(qh, BL * V)
                    sync.dma_start(out=qt_t[:, sl], in_=qt[:, sl]).then_inc(
                        sem_qt, 16
                    )
                elif kind == "wd":
                    sync.dma_start(out=wd_t[:], in_=wdiag[:]).then_inc(sem_wd, 16)
                elif kind == "wg":
                    sync.wait_ge(sem_g, tok[1])
            sync.wait_ge(sem_dve, 2)
            sync.wait_ge(sem_act, 3)
            sync.dma_start(out=po[:], in_=acc_t[:]).then_inc(sem_out, 16)
            sync.wait_ge(sem_out, 16)

        @block.gpsimd
        def _(gpsimd):
            gpsimd.wait_ge(sem_cp, 16)  # mapping loaded
            for b in range(BL):
                gpsimd.wait_ge(sem_neg, 16 * (_chunk_of(b) + 1))
                gpsimd.indirect_dma_start(
                    out=d_t[:, b * K : (b + 1) * K],
                    out_offset=None,
                    in_=gath[:],
                    in_offset=bass.IndirectOffsetOnAxis(
                        ap=map_i[:, b : b + 1], axis=0
                    ),
                    compute_op=Alu.add,
                ).then_inc(sem_g, 16)

        # single accumulation group across the whole bank: only the very
        # first matmul has start=True, only the very last has stop=True
        first_mm = [True]
        DR = mybir.MatmulPerfMode.DoubleRow

        def mm(out, lhsT, rhs, stop=False, dr=False):
            inst = nc.tensor.matmul(
                out=out,
                lhsT=lhsT,
                rhs=rhs,
                start=first_mm[0],
                stop=stop,
                perf_mode=DR if dr else None,
                skip_group_check=True,
            )
            first_mm[0] = False
            return inst

        def gram(region, a, b=None):
            # square-gram of columns `a` (or cross-gram a x b) into `region`;
            # USE_DR pairs two k-tiles per pass (DoubleRow)
            if USE_DR:
                w = a.shape[-1] // 2
                a2 = a.rearrange("s (t c) -> s t c", t=2)
                b2 = a2 if b is None else b.rearrange("s (t c) -> s t c", t=2)
                return mm(ps[0:w, region : region + w], a2, b2, dr=True)
            w = a.shape[-1]
            return mm(ps[0:w, region : region + w], a, a if b is None else b)

        def d_grams(tensor, b):
            tensor.wait_ge(sem_g, 16 * (b + 1))
            rw = 256 if USE_DR else 128
            for i in range(D // rw):
                gram(R_RECON, d3[:, b, i * rw : (i + 1) * rw])
            if USE_DR:
                gram(R_PTS, d3[:, b, D : D + PC])
            else:
                gram(R_PTS, d3[:, b, D : D + 128])
                gram(R_PTS, d3[:, b, D + 128 : D + PC])

        @block.tensor
        def _(tensor):
            for b in range(6):
                d_grams(tensor, b)
            # kl grams: ln halves are done by the time batch 5 is gathered
            kw = 256 if USE_DR else 128
            tensor.wait_ge(sem_act, 2)
            for c in range(BL * V // kw):
                gram(
                    R_KL,
                    qt_t[:, c * kw : (c + 1) * kw],
                    l_t[:, c * kw : (c + 1) * kw],
                )
            for b in range(6, BL):
                d_grams(tensor, b)
            # markx: cols D..D+16 of every batch -> out rows b*16+c;
            # wdiag keeps only (p % 16) < 8 (the landmark cols)
            mm(
                ps[:, R_MARK : R_MARK + 128],
                d3[:, :, D : D + 16],
                d3[:, :, D : D + 16],
                stop=True,
            ).then_inc(sem_pe, 1)

        @block.scalar
        def _(scalar):
            scalar.wait_ge(sem_cp, 16)
            scalar.wait_ge(sem_qt, 16)
            nc.scalar.activation(
                l_t[:, : BL // 2 * V],
                qt_t[:, : BL // 2 * V],
                Act.Ln,
                bias=cp_t[:, COL_BIAS : COL_BIAS + 1],
                scale=1.0,
            ).then_inc(sem_act, 1)
            scalar.wait_ge(sem_qt, 32)
            nc.scalar.activation(
                l_t[:, BL // 2 * V :],
                qt_t[:, BL // 2 * V :],
                Act.Ln,
                bias=cp_t[:, COL_BIAS : COL_BIAS + 1],
                scale=1.0,
            ).then_inc(sem_act, 1)
            # best term: acc[0:118, 1] = per-partition sum(bd^2), in place
            scalar.wait_ge(sem_dve, 1)
            nc.scalar.activation(
                bd_t[0:P, :],
                bd_t[0:P, :],
                Act.Square,
                accum_out=acc_t[0:P, 1:2],
            ).then_inc(sem_act, 1)

        @block.vector
        def _(vector):
            # best diff: bd = sqrt(w)*(best - best_gt), f32
            vector.wait_ge(sem_cp, 16)
            nc.vector.tensor_sub(
                bd_t[:],
                cp_t[:, COL_BEST : COL_BEST + BC],
                cp_t[:, COL_BESTGT : COL_BESTGT + BC],
            ).then_inc(sem_dve, 1)  # 1
            # fused extraction: (psum * 1.0) * wdiag, per-partition accumulate
            vector.wait_ge(sem_wd, 16)
            vector.wait_ge(sem_pe, 1)
            nc.vector.scalar_tensor_tensor(
                out=ext_t[:],
                in0=ps[:, :],
                scalar=1.0,
                in1=wd_t[:],
                op0=Alu.mult,
                op1=Alu.mult,
                accum_out=acc_t[:, 0:1],
            ).then_inc(sem_dve, 1)  # 2

    return nc


def _get_nc(vector_dims: int):
    key = ("nc", vector_dims)
    if key not in _CACHE:
        _CACHE[key] = _build_bass(vector_dims)
    return _CACHE[key]


def _prepare(inputs):
    import ml_dtypes

    fp8 = ml_dtypes.float8_e4m3

    zs = np.asarray(inputs["zs"], dtype=np.float32)
    rzs = np.asarray(inputs["rzs"], dtype=np.float32)
    pts = np.asarray(inputs["pts"], dtype=np.float32)
    pts_gt = np.asarray(inputs["pts_gt"], dtype=np.float32)
    qy = np.asarray(inputs["qy"], dtype=np.float32)
    best = np.asarray(inputs["best"], dtype=np.float64)
    best_gt = np.asarray(inputs["best_gt"], dtype=np.float64)
    mapping = np.asarray(inputs["mapping"])
    vector_dims = int(np.asarray(inputs["vector_dims"]))

    # landmark P-indices permuted to the front of the P axis
    perm = list(MARKS) + [p for p in range(P) if p not in MARKS]
    pts_p = pts[:, :, perm, :].reshape(B, S, PC)
    ptsgt_p = pts_gt[:, :, perm, :].reshape(B, S, PC)

    neg_b = np.empty((B, S, K), dtype=fp8)
    neg_b[:, :, :D] = (-zs).astype(fp8)
    neg_b[:, :, D:] = (-ptsgt_p).astype(fp8)
    gath_b = np.empty((B, S, K), dtype=fp8)
    gath_b[:, :, :D] = rzs.astype(fp8)
    gath_b[:, :, D:] = pts_p.astype(fp8)
    qt_b = (qy * vector_dims).astype(fp8)

    # sqrt of landmark weights for the best term (exact in f64)
    w_p = np.ones(P, dtype=np.float64)
    w_p[list(MARKS)] += W_MARK
    w_sq = np.sqrt(w_p)
    best_w = (best * w_sq[None, :, None]).astype(np.float32)
    bestgt_w = (best_gt * w_sq[None, :, None]).astype(np.float32)

    # weighted diagonal extraction mask (term normalizations baked in)
    wd = np.zeros((S, 512), dtype=np.float32)
    ii = np.arange(128)
    wd[ii, R_RECON + ii] = GAMMA / (B * S * D)
    wd[ii, R_PTS + ii] = 1.0 / (B * S * PC)
    wd[ii, R_KL + ii] = BETA / (vector_dims * B * S)
    # markx out row p = b*16 + c with c in 0..16; marks are c < 8
    wmark = np.where((ii % 16) < NMARK * C, W_MARK / (B * S * PC), 0.0)
    wd[ii, R_MARK + ii] = wmark

    base = (np.arange(BL, dtype=np.int32) * S)[:, None]

    in_maps = []
    for c in range(N_CORES):
        sl = slice(c * BL, (c + 1) * BL)
        map_abs = np.ascontiguousarray(
            (mapping[sl].astype(np.int32) + base).T
        )  # (S, BL)
        cpk = np.zeros((S, NCONST), dtype=np.float32)
        cpk[:, COL_MAP : COL_MAP + BL] = map_abs.view(np.float32)
        cpk[:, COL_BIAS] = np.float32(vector_dims * EPS)
        cpk[:P, COL_BEST : COL_BEST + BC] = (
            best_w[sl].transpose(1, 0, 2).reshape(P, BC)
        )
        cpk[:P, COL_BESTGT : COL_BESTGT + BC] = (
            bestgt_w[sl].transpose(1, 0, 2).reshape(P, BC)
        )
        in_maps.append(
            {
                # pre-transposed to the SBUF layout [S, BL*K]
                "neg": np.ascontiguousarray(
                    neg_b[sl].transpose(1, 0, 2).reshape(S, BL * K)
                ),
                "gath": gath_b[sl].reshape(BL * S, K),
                "qt": np.ascontiguousarray(
                    qt_b[sl].transpose(1, 0, 2).reshape(S, BL * V)
                ),
                "cpack": cpk,
                "wdiag": wd,
            }
        )
    return in_maps, vector_dims


def _combine(results) -> np.ndarray:
    total = np.float64(0.0)
    for r in results:
        por = r["po"].astype(np.float64)
        total += por[:, 0].sum()  # weighted recon+pts+kl+marks
        total += por[:P, 1].sum() / (B * PC)  # best
    return np.float32(total)


def kernel(**inputs) -> np.ndarray:
    from concourse.bass_utils import run_bass_kernel_spmd

    in_maps, vector_dims = _prepare(inputs)
    nc = _get_nc(vector_dims)

    trace = os.environ.get("KERNEL_TRACE", "") == "1"
    res = run_bass_kernel_spmd(nc, in_maps, core_ids=list(range(N_CORES)), trace=trace)
    if trace and res.exec_time_ns is not None:
        print(f"HW exec time: {res.exec_time_ns} ns")
        if res.instructions_and_trace is not None:
            print(f"trace: {res.instructions_and_trace[1]}")

    return _combine(res.results)
